# revision 1
# baseline (speedup 1.0000x reference)
"""Trainium2 Bass kernel for packed-segment causal GQA attention.

Shapes (hardcoded): x [4096, 2048], 16 q heads / 4 kv heads, head dim 128,
4 packed segments of 1024 tokens.

Sharding over 8 cores: core c -> segment c//2 (tokens), head-half c%2
(8 q heads + their 2 kv heads; wq/wk/wv column-sharded, wo row-sharded).
Each core computes a partial out^T [2048, 1024] for its segment; host sums
the two partials per segment (wo row-shard) and transposes back.

On-device dataflow (all in transposed token space, bf16 matmuls, fp32 PSUM):
  q^T = wq_sh^T @ x_seg^T            (lhsT = wq_sh, rhs = x^T)
  RoPE via pair-swap matmul + cos/sin elementwise combine
  s^T[key, row] = k^T_tile^T.T @ q^T  per 128-key x 512-row block
  p^T = exp(s^T/sqrt(d)) with additive triangular mask on diagonal chunks
  denom: key-block dim collapsed on DVE (reduce over the causal extent
    only), then ONE ones-matmul per row group in f32r (1 cyc/row) gives
    the partition-broadcast key sum -- 8.2K PE cycles vs 36.9K for the
    per-block ones-matmul accumulation
  o^T += v_tile.T @ p^T (v kept in [token, d] layout from projection)
  a^T = o^T * (1/denom);  out^T = wo_sh^T @ a^T (bf16 DMA out)
"""

import os
import re

import numpy as np
import ml_dtypes

import bass_rust
import concourse.bass as bass
import concourse.mybir as mybir
import concourse.tile as tile
from concourse.bass_utils import run_bass_kernel_spmd
from concourse.vector_clock import ScopedClock

BF16 = ml_dtypes.bfloat16
F32 = mybir.dt.float32
BF = mybir.dt.bfloat16

DIM, H, HKV, D, B, S = 2048, 16, 4, 128, 4, 1024
REP = H // HKV
SCALE = float(D) ** -0.5
NCORE = 8
HLOC = 8          # q heads per core
GLOC = 2          # kv heads per core


_MAXW = 1


def _patch_wait_split(tilemod):
    """walrus in this env caps sem waits per instruction: rewrite any
    instruction carrying more than _MAXW waits so the excess waits land on
    same-engine NoOps inserted just before it."""

    orig_lower = tilemod.TileContext._lower_ordered_insts

    def _split_block(nc, insts):
        out = []
        for inst in insts:
            si = getattr(inst, "sync_info", None)
            waits = list(si.on_wait) if si is not None and si.on_wait else []
            if len(waits) > _MAXW:
                head, rest = waits[:-_MAXW], waits[-_MAXW:]
                for w in head:  # NoOp is CTRL-class: keep it to 1 wait each
                    out.append(
                        mybir.InstNoOp(
                            name=nc.get_next_instruction_name(),
                            engine=inst.engine,
                            bass_nofuse=True,
                            debug=inst.debug,
                            sync_info=mybir.SyncInfo(on_wait=[w], on_update=[]),
                        )
                    )
                inst.sync_info = mybir.SyncInfo(
                    on_wait=rest, on_update=list(si.on_update)
                )
            out.append(inst)
        insts[:] = out

    def patched(self, ordered):
        for insts in ordered.values():
            _split_block(self.nc, insts)
        return orig_lower(self, ordered)

    tilemod.TileContext._lower_ordered_insts = patched


def _patch_drain(tilemod):
    """walrus in this env rejects >1 sem wait on CTRL instructions: split the
    TileContext-exit drain's waits across single-wait SP NoOPs."""

    def _drain_and_barrier_split(self, tick_clock, wait_clock):
        nc = self.nc
        gc = tick_clock.global_clock
        ticks = [int(t) for t in re.findall(r"\d+", str(gc))]
        for idx, tick in enumerate(ticks):
            if tick <= 0:
                continue
            part = bass_rust.VectorClock()
            part.require_at_least(idx, tick)
            n = nc.sync.nop(hint="drain_split", nofuse=True)
            wait_clock.add_sem_waits(n.ins, ScopedClock({None: part}))
        d = nc.sync.drain()
        wait_clock.add_sem_waits(
            d.ins, ScopedClock({None: gc}), cur_clock=ScopedClock({None: gc})
        )
        nc.all_engine_barrier()
        assert self.sems is not None
        popped = nc._tile_sem_poison_stack.pop()
        assert popped is self._sem_poison
        nc.clear_and_free_semaphores(list(self.sems.allocated().values()))
        nc.all_engine_barrier()

    tilemod.TileContext._drain_and_barrier = _drain_and_barrier_split


_patch_wait_split(tile)
_patch_drain(tile)

_PROGRAM = None


def _build_program():
    nc = bass.Bass()

    xT = nc.declare_dram_parameter("xT", [DIM, S], BF, isOutput=False)
    wq = nc.declare_dram_parameter("wq", [DIM, HLOC * D], BF, isOutput=False)
    # wk/wv host-rearranged to partition-major [128, KC*256]: 8KB DMA lines
    # run at full queue rate; the natural [128,kc,256] layout has 512B lines
    # at ~40% of that, which starved the projections
    wkf = nc.declare_dram_parameter("wkf", [128, (DIM // 128) * GLOC * D], BF, isOutput=False)
    wvf = nc.declare_dram_parameter("wvf", [128, (DIM // 128) * GLOC * D], BF, isOutput=False)
    wo = nc.declare_dram_parameter("wo", [HLOC * D, DIM], BF, isOutput=False)
    cosT = nc.declare_dram_parameter("cosT", [D, S], BF, isOutput=False)
    sinT = nc.declare_dram_parameter("sinT", [D, S], BF, isOutput=False)
    swp = nc.declare_dram_parameter("swp", [D, D], BF, isOutput=False)
    tri = nc.declare_dram_parameter("tri", [D, D], F32, isOutput=False)
    outT = nc.declare_dram_parameter("outT", [DIM, S], BF, isOutput=True)

    xT_r = xT.rearrange("(o p) t -> p o t", p=128)      # [128, 16, 1024]
    wq_r = wq.rearrange("(o p) f -> p o f", p=128)      # [128, 16, 1024]
    wo_r = wo.rearrange("(c p) e -> p c e", p=128)      # [128, 8, 2048]
    outT_r = outT.rearrange("(o p) t -> p o t", p=128)  # [128, 16, 1024]

    KC = DIM // 128      # 16 contraction chunks
    NTB = S // 512       # 2 token blocks of 512
    NKB = S // 128       # 8 key blocks of 128

    with tile.TileContext(nc) as tc:
        with (
            tc.tile_pool(name="consts", bufs=1) as consts,
            tc.tile_pool(name="rope", bufs=2) as rope_pool,
            tc.tile_pool(name="pt", bufs=2) as pt_pool,
            tc.tile_pool(name="dinv", bufs=2) as dinv_pool,
            tc.tile_pool(name="ostage", bufs=3) as ostage,
            tc.tile_pool(name="psA", bufs=2, space="PSUM") as psA,
            tc.tile_pool(name="psS", bufs=3, space="PSUM") as psS,
            tc.tile_pool(name="psO", bufs=2, space="PSUM") as psO,
            tc.tile_pool(name="psD", bufs=1, space="PSUM") as psD,
        ):
            # ---- constant loads (per-chunk so the first matmuls start early) ----
            xT_sb = consts.tile([128, KC, S], BF)
            wk_sb = consts.tile([128, KC, GLOC * D], BF)
            wq_sb = consts.tile([128, KC, HLOC * D], BF)
            wv_sb = consts.tile([128, KC, GLOC * D], BF)
            # Two parallel dispatch streams (~650ns SP/ACT sequencer cost per
            # dma_start, one queue per dispatch): SP streams the xT chunks in
            # consumption order (chunk 0 partition-split so the first matmul
            # starts ~3us earlier); ACT dispatches wk/wv flats + constants.
            # wq/wo follow on SP AFTER the xT wave - dispatching them early
            # floods the shared DMA queues and starves the projections.
            for qs in range(4):
                pp = slice(qs * 32, (qs + 1) * 32)
                nc.sync.dma_start(out=xT_sb[pp, 0, :], in_=xT_r[pp, 0, :])
            for kc in range(1, KC):
                nc.sync.dma_start(out=xT_sb[:, kc, :], in_=xT_r[:, kc, :])
            wk_flat = wk_sb.rearrange("p a b -> p (a b)")
            wv_flat = wv_sb.rearrange("p a b -> p (a b)")
            for qs in range(4):
                pp = slice(qs * 32, (qs + 1) * 32)
                nc.scalar.dma_start(out=wk_flat[pp, :], in_=wkf[pp, :])
            for qs in range(4):
                pp = slice(qs * 32, (qs + 1) * 32)
                nc.scalar.dma_start(out=wv_flat[pp, :], in_=wvf[pp, :])
            swp_sb = consts.tile([128, D], BF)
            nc.scalar.dma_start(out=swp_sb, in_=swp[:, :])
            cos_sb = consts.tile([128, S], BF)
            nc.scalar.dma_start(out=cos_sb, in_=cosT[:, :])
            sin_sb = consts.tile([128, S], BF)
            nc.scalar.dma_start(out=sin_sb, in_=sinT[:, :])
            tri_sb = consts.tile([128, D], F32)
            nc.scalar.dma_start(out=tri_sb, in_=tri[:, :])
            for kc in range(KC):
                nc.sync.dma_start(out=wq_sb[:, kc, :], in_=wq_r[:, kc, :])
            wo_sb = consts.tile([128, HLOC, DIM], BF)
            for c in range(HLOC):
                nc.sync.dma_start(out=wo_sb[:, c, :], in_=wo_r[:, c, :])

            # all-ones stationary operand: the denom matmul then writes the
            # key-sum to EVERY output partition (free partition-broadcast)
            ones_sq = consts.tile([128, 128], BF)
            nc.vector.memset(ones_sq, 1.0)

            # persistent activations
            qT_sb = consts.tile([128, HLOC, S], BF)   # q^T, rotated
            kT_sb = consts.tile([128, GLOC, S], BF)   # k^T, rotated
            v_sb = consts.tile([128, NKB, GLOC * D], BF)  # v in [tok, d]
            aT_sb = consts.tile([128, HLOC, S], BF)   # attention out^T

            def rope_finish(ps, tb, dst_sb, dst_idx):
                """dst[:, dst_idx, tb*512:+512] = rope(ps) (ps = w^T@x^T)."""
                qsb = rope_pool.tile([128, 512], BF, tag="qsb")
                nc.scalar.copy(out=qsb, in_=ps)
                ps2 = psS.tile([128, 512], F32, tag="psS")
                nc.tensor.matmul(ps2, swp_sb, qsb, start=True, stop=True)
                tspan = slice(tb * 512, (tb + 1) * 512)
                t1 = rope_pool.tile([128, 512], F32, tag="t1")
                nc.vector.tensor_mul(out=t1, in0=qsb, in1=cos_sb[:, tspan])
                t2 = rope_pool.tile([128, 512], F32, tag="t2")
                nc.vector.tensor_mul(out=t2, in0=ps2, in1=sin_sb[:, tspan])
                nc.vector.tensor_add(
                    out=dst_sb[:, dst_idx, tspan], in0=t1, in1=t2
                )

            def proj_rope(w_sb, hd_idx, tb, dst_sb, dst_idx):
                ps = psA.tile([128, 512], F32, tag="ps")
                for kc in range(KC):
                    nc.tensor.matmul(
                        ps,
                        w_sb[:, kc, hd_idx * 128:(hd_idx + 1) * 128],
                        xT_sb[:, kc, tb * 512:(tb + 1) * 512],
                        start=(kc == 0),
                        stop=(kc == KC - 1),
                    )
                rope_finish(ps, tb, dst_sb, dst_idx)

            # ---- kc-outer K+V projection ----
            # One pass over the xT chunks with all 12 accumulators (4 K tiles
            # + 8 V halves paired two-per-bank) live across the contraction:
            # ~2us of matmul per chunk matches the ~1.3us DMA chunk arrival
            # rate, where the old per-tile loops burned a full 16-chunk pass
            # in 3.8us and stalled ~12us waiting on the xT wave.
            kps = [
                psA.tile([128, 512], F32, tag="ps", name="kps0"),
                psA.tile([128, 512], F32, tag="ps", name="kps1"),
                psS.tile([128, 512], F32, tag="psS", name="kps2"),
                psS.tile([128, 512], F32, tag="psS", name="kps3"),
            ]
            for kc in range(KC):
                for g in range(GLOC):
                    for tb in range(NTB):
                        nc.tensor.matmul(
                            kps[2 * g + tb],
                            wk_sb[:, kc, g * 128:(g + 1) * 128],
                            xT_sb[:, kc, tb * 512:(tb + 1) * 512],
                            start=(kc == 0),
                            stop=(kc == KC - 1),
                        )
            rope_finish(kps[2], 0, kT_sb, 1)
            rope_finish(kps[3], 1, kT_sb, 1)
            rope_finish(kps[0], 0, kT_sb, 0)
            rope_finish(kps[1], 1, kT_sb, 0)
            for vtb in range(NKB):
                ps = psA.tile([128, 512], F32, tag="ps")
                for kc in range(KC):
                    nc.tensor.matmul(
                        ps[:, : GLOC * D],
                        xT_sb[:, kc, vtb * 128:(vtb + 1) * 128],
                        wv_sb[:, kc, :],
                        start=(kc == 0),
                        stop=(kc == KC - 1),
                    )
                nc.scalar.copy(out=v_sb[:, vtb, :], in_=ps[:, : GLOC * D])

            def attention(h):
                g = h // REP
                for rg in range(NTB):
                    rows = slice(rg * 512, (rg + 1) * 512)
                    pt = pt_pool.tile([128, NKB, 512], BF, tag="pt")
                    po = psO.tile([128, 512], F32, tag="psO")
                    pd = psD.tile([128, 512], F32, tag="psD")
                    nkb = 4 * rg + 4

                    def span_of(kb):
                        return slice(max(0, kb - 4 * rg) * 128, 512)

                    def score_exp(kb):
                        c0 = max(0, kb - 4 * rg)
                        span = span_of(kb)
                        ps = psS.tile([128, 512], F32, tag="psS", name="ps")
                        nc.tensor.matmul(
                            ps[:, span],
                            kT_sb[:, g, kb * 128:(kb + 1) * 128],
                            qT_sb[:, h, rg * 512 + c0 * 128:(rg + 1) * 512],
                            start=True,
                            stop=True,
                        )
                        if kb - 4 * rg >= 0:
                            cc = kb - 4 * rg
                            nc.vector.tensor_add(
                                out=ps[:, cc * 128:(cc + 1) * 128],
                                in0=ps[:, cc * 128:(cc + 1) * 128],
                                in1=tri_sb,
                            )
                        nc.scalar.activation(
                            out=pt[:, kb, span],
                            in_=ps[:, span],
                            func=mybir.ActivationFunctionType.Exp,
                            scale=SCALE,
                        )

                    # scores issued 2 key-blocks ahead (matches the 3 psS
                    # buffers) so exp(kb) is done before pd/po(kb) issue -
                    # removes ~0.3-0.5us PE waits per block
                    score_exp(0)
                    if nkb > 1:
                        score_exp(1)
                    for kb in range(nkb):
                        if kb + 2 < nkb:
                            score_exp(kb + 2)
                        span = span_of(kb)
                        nc.tensor.matmul(
                            pd[:, span],
                            ones_sq,
                            pt[:, kb, span],
                            start=(kb == 0),
                            stop=(kb == nkb - 1),
                        )
                        nc.tensor.matmul(
                            po[:, span],
                            v_sb[:, kb, g * D:(g + 1) * D],
                            pt[:, kb, span],
                            start=(kb == 0),
                            stop=(kb == nkb - 1),
                        )
                    # 1/denom as exp(-ln(denom)) on ACT: DVE reciprocal costs
                    # ~6.5ns/elem (3.3us/tile) and was stalling the PE
                    lnd = dinv_pool.tile([128, 512], F32, tag="lnd")
                    nc.scalar.activation(
                        out=lnd, in_=pd, func=mybir.ActivationFunctionType.Ln
                    )
                    dinv_b = dinv_pool.tile([128, 512], F32, tag="dinvb")
                    nc.scalar.activation(
                        out=dinv_b, in_=lnd,
                        func=mybir.ActivationFunctionType.Exp, scale=-1.0,
                    )
                    nc.vector.tensor_mul(
                        out=aT_sb[:, h, rows],
                        in0=po,
                        in1=dinv_b,
                    )

            # ---- per-head: project q for head h+1, attend head h ----
            for tb in range(NTB):
                proj_rope(wq_sb, 0, tb, qT_sb, 0)
            for h in range(HLOC):
                if h + 1 < HLOC:
                    for tb in range(NTB):
                        proj_rope(wq_sb, h + 1, tb, qT_sb, h + 1)
                attention(h)

            # ---- output projection: out^T = wo_sh^T @ a^T ----
            for et in range(KC):
                for tb in range(NTB):
                    ps = psA.tile([128, 512], F32, tag="ps")
                    for c in range(HLOC):
                        nc.tensor.matmul(
                            ps,
                            wo_sb[:, c, et * 128:(et + 1) * 128],
                            aT_sb[:, c, tb * 512:(tb + 1) * 512],
                            start=(c == 0),
                            stop=(c == HLOC - 1),
                        )
                    st = ostage.tile([128, 512], BF, tag="st")
                    nc.vector.tensor_copy(out=st, in_=ps)
                    nc.sync.dma_start(
                        out=outT_r[:, et, tb * 512:(tb + 1) * 512], in_=st
                    )

    return nc


LAST_RESULT = None
_TRACE = os.environ.get("BASS_ATTN_TRACE", "") == "1"

if _TRACE:
    # Register the NTFF profile hook that the agent image's antenv lacks
    # (test/profiling only; the graded path never enters this branch).
    try:
        import sys
        import types

        import antenv  # noqa: F401

        if "antenv.axon_hooks" not in sys.modules:
            _mod = types.ModuleType("antenv.axon_hooks")
            _hook_box = [None]
            _mod.set_axon_ntff_profile_hook = lambda h: _hook_box.__setitem__(0, h)
            _mod.get_axon_ntff_profile_hook = lambda: _hook_box[0]
            sys.modules["antenv.axon_hooks"] = _mod
            import antenv as _antenv

            _antenv.axon_hooks = _mod
            from trn_agent_boot.trn_boot import _ntff_profile_via_ctypes

            _mod.set_axon_ntff_profile_hook(
                _ntff_profile_via_ctypes("/opt/axon/libaxon_pjrt.so")
            )
    except Exception as e:  # pragma: no cover
        print(f"NTFF hook setup failed ({e}); tracing will be skipped")


def kernel(x, freqs_cis, wq, wk, wv, wo, seq_len=None, **_ignored):
    global _PROGRAM, LAST_RESULT
    x = np.ascontiguousarray(np.asarray(x, dtype=np.float32))
    fc = np.asarray(freqs_cis, dtype=np.float32)
    wq = np.asarray(wq, dtype=np.float32)
    wk = np.asarray(wk, dtype=np.float32)
    wv = np.asarray(wv, dtype=np.float32)
    wo = np.asarray(wo, dtype=np.float32)

    # host-side prep (sharding + transposed/bf16 views + rope/mask constants)
    xT = np.ascontiguousarray(x.T).astype(BF16)                    # [2048, 4096]
    cos = np.ascontiguousarray(np.repeat(fc[:S, :, 0], 2, axis=1).T).astype(BF16)
    sgn = np.where(np.arange(D) % 2 == 0, -1.0, 1.0).astype(np.float32)
    sin = np.ascontiguousarray(
        (np.repeat(fc[:S, :, 1], 2, axis=1) * sgn[None, :]).T
    ).astype(BF16)
    swp = np.zeros((D, D), BF16)
    swp[np.arange(D), np.arange(D) ^ 1] = 1
    k_idx = np.arange(128)[:, None]
    r_idx = np.arange(128)[None, :]
    tri = np.where(r_idx >= k_idx, 0.0, -1e9).astype(np.float32)

    def pflat(w):
        # partition-major flat layout: out[p, kc*F+f] = w[kc*128+p, f]
        return np.ascontiguousarray(
            w.reshape(16, 128, -1).transpose(1, 0, 2).reshape(128, -1)
        )

    in_maps = []
    for c in range(NCORE):
        s, h2 = c // 2, c % 2
        in_maps.append(
            {
                "xT": np.ascontiguousarray(xT[:, s * S:(s + 1) * S]),
                "wq": wq[:, h2 * HLOC * D:(h2 + 1) * HLOC * D].astype(BF16),
                "wkf": pflat(wk[:, h2 * GLOC * D:(h2 + 1) * GLOC * D].astype(BF16)),
                "wvf": pflat(wv[:, h2 * GLOC * D:(h2 + 1) * GLOC * D].astype(BF16)),
                "wo": wo[h2 * HLOC * D:(h2 + 1) * HLOC * D, :].astype(BF16),
                "cosT": cos,
                "sinT": sin,
                "swp": swp,
                "tri": tri,
            }
        )

    if _PROGRAM is None:
        _PROGRAM = _build_program()

    res = run_bass_kernel_spmd(
        _PROGRAM, in_maps, core_ids=list(range(NCORE)), trace=_TRACE
    )
    LAST_RESULT = res

    out = np.empty((B * S, DIM), np.float32)
    for s in range(B):
        outT = (
            res.results[2 * s]["outT"].astype(np.float32)
            + res.results[2 * s + 1]["outT"].astype(np.float32)
        )
        out[s * S:(s + 1) * S, :] = outT.T
    return out



# revision 4
# speedup vs baseline: 1.1348x; 1.1348x over previous
"""Trainium2 Bass kernel for packed-segment causal GQA attention.

Shapes (hardcoded): x [4096, 2048], 16 q heads / 4 kv heads, head dim 128,
4 packed segments of 1024 tokens.

Sharding over 8 cores: core c -> segment c//2 (tokens), head-half c%2
(8 q heads + their 2 kv heads; wq/wk/wv column-sharded, wo row-sharded).
Each core computes a partial out^T [2048, 1024] for its segment; host sums
the two partials per segment (wo row-shard) and transposes back.

On-device dataflow (all in transposed token space, bf16 matmuls, fp32 PSUM):
  joint kc-outer pass: k^T (4 tiles) + q0^T (2 tiles) accumulate across the
    16 contraction chunks as the xT chunks stream in, so the PE tracks the
    DMA wave instead of waiting for full tensors
  RoPE via pair-swap matmul + cos/sin elementwise combine
  v in [token, d] layout (vtb-outer after xT completes)
  s^T[key, row] = k^T_tile^T.T @ q^T  per 128-key x 512-row block
  p^T = exp(s^T/sqrt(d)) with additive triangular mask on diagonal chunks
  denom: key-block dim collapsed on DVE (bf16, causal extents only), then
    ONE ones-matmul per row group broadcasts the key-sum to all partitions
    (~8.2K PE cycles vs 36.9K for per-block ones-matmul accumulation)
  o^T += v_tile.T @ p^T;  a^T = o^T * (1/denom) with 1/denom = exp(-ln(d))
  out^T = wo_sh^T @ a^T, evacuated PSUM->SBUF on GPSIMD, DMA out on SP

DMA plan: SP carries xT (fine-grained chunks first) then wq[h1..7] then wo;
ACT carries wk/wq-head0 flats (chunk-grouped, so the kc-outer pass starts as
soon as chunk group 0 lands) then rope constants then wv. Dispatches are
batched into multi-chunk APs to amortize the ~650ns sequencer cost each.
"""

import os
import re

import numpy as np
import ml_dtypes

import bass_rust
import concourse.bass as bass
import concourse.mybir as mybir
import concourse.tile as tile
from concourse.bass_utils import run_bass_kernel_spmd
from concourse.vector_clock import ScopedClock

BF16 = ml_dtypes.bfloat16
F32 = mybir.dt.float32
BF = mybir.dt.bfloat16

DIM, H, HKV, D, B, S = 2048, 16, 4, 128, 4, 1024
REP = H // HKV
SCALE = float(D) ** -0.5
NCORE = 8
HLOC = 8          # q heads per core
GLOC = 2          # kv heads per core


_MAXW = 1


def _patch_wait_split(tilemod):
    """walrus in this env caps sem waits per instruction: rewrite any
    instruction carrying more than _MAXW waits so the excess waits land on
    same-engine NoOps inserted just before it."""

    orig_lower = tilemod.TileContext._lower_ordered_insts

    def _split_block(nc, insts):
        out = []
        for inst in insts:
            si = getattr(inst, "sync_info", None)
            waits = list(si.on_wait) if si is not None and si.on_wait else []
            if len(waits) > _MAXW:
                head, rest = waits[:-_MAXW], waits[-_MAXW:]
                for w in head:  # NoOp is CTRL-class: keep it to 1 wait each
                    out.append(
                        mybir.InstNoOp(
                            name=nc.get_next_instruction_name(),
                            engine=inst.engine,
                            bass_nofuse=True,
                            debug=inst.debug,
                            sync_info=mybir.SyncInfo(on_wait=[w], on_update=[]),
                        )
                    )
                inst.sync_info = mybir.SyncInfo(
                    on_wait=rest, on_update=list(si.on_update)
                )
            out.append(inst)
        insts[:] = out

    def patched(self, ordered):
        for insts in ordered.values():
            _split_block(self.nc, insts)
        return orig_lower(self, ordered)

    tilemod.TileContext._lower_ordered_insts = patched


def _patch_drain(tilemod):
    """walrus in this env rejects >1 sem wait on CTRL instructions: split the
    TileContext-exit drain's waits across single-wait SP NoOPs."""

    def _drain_and_barrier_split(self, tick_clock, wait_clock):
        nc = self.nc
        gc = tick_clock.global_clock
        ticks = [int(t) for t in re.findall(r"\d+", str(gc))]
        for idx, tick in enumerate(ticks):
            if tick <= 0:
                continue
            part = bass_rust.VectorClock()
            part.require_at_least(idx, tick)
            n = nc.sync.nop(hint="drain_split", nofuse=True)
            wait_clock.add_sem_waits(n.ins, ScopedClock({None: part}))
        d = nc.sync.drain()
        wait_clock.add_sem_waits(
            d.ins, ScopedClock({None: gc}), cur_clock=ScopedClock({None: gc})
        )
        nc.all_engine_barrier()
        assert self.sems is not None
        popped = nc._tile_sem_poison_stack.pop()
        assert popped is self._sem_poison
        nc.clear_and_free_semaphores(list(self.sems.allocated().values()))
        nc.all_engine_barrier()

    tilemod.TileContext._drain_and_barrier = _drain_and_barrier_split


_patch_wait_split(tile)
_patch_drain(tile)

_PROGRAM = None


def _build_program():
    nc = bass.Bass()

    xT = nc.declare_dram_parameter("xT", [DIM, S], BF, isOutput=False)
    # head-0 q weights, partition-major flat [128, 16*128] (chunk kc at cols
    # kc*128) so the h0 projection can join the kc-outer pass; heads 1-7 in
    # the natural column layout (chunk DMAs there have 1792B lines)
    wqh0 = nc.declare_dram_parameter("wqh0", [128, (DIM // 128) * D], BF, isOutput=False)
    wqr = nc.declare_dram_parameter("wqr", [DIM, (HLOC - 1) * D], BF, isOutput=False)
    # wk/wv host-rearranged to partition-major [128, KC*256]: big DMA lines
    # run at full queue rate; the natural [128,kc,256] layout has 512B lines
    # at ~40% of that, which starved the projections
    wkf = nc.declare_dram_parameter("wkf", [128, (DIM // 128) * GLOC * D], BF, isOutput=False)
    wvf = nc.declare_dram_parameter("wvf", [128, (DIM // 128) * GLOC * D], BF, isOutput=False)
    wo = nc.declare_dram_parameter("wo", [HLOC * D, DIM], BF, isOutput=False)
    cosT = nc.declare_dram_parameter("cosT", [D, S], BF, isOutput=False)
    sinT = nc.declare_dram_parameter("sinT", [D, S], BF, isOutput=False)
    swp = nc.declare_dram_parameter("swp", [D, D], BF, isOutput=False)
    tri = nc.declare_dram_parameter("tri", [D, D], F32, isOutput=False)
    outT = nc.declare_dram_parameter("outT", [DIM, S], BF, isOutput=True)

    xT_r = xT.rearrange("(o p) t -> p o t", p=128)      # [128, 16, 1024]
    wqr_r = wqr.rearrange("(o p) f -> p o f", p=128)    # [128, 16, 896]
    wo_r = wo.rearrange("(c p) e -> p c e", p=128)      # [128, 8, 2048]
    outT_r = outT.rearrange("(o p) t -> p o t", p=128)  # [128, 16, 1024]

    KC = DIM // 128      # 16 contraction chunks
    NTB = S // 512       # 2 token blocks of 512
    NKB = S // 128       # 8 key blocks of 128

    with tile.TileContext(nc) as tc:
        with (
            tc.tile_pool(name="consts", bufs=1) as consts,
            tc.tile_pool(name="rope", bufs=2) as rope_pool,
            tc.tile_pool(name="pt", bufs=2) as pt_pool,
            tc.tile_pool(name="acc", bufs=2) as acc_pool,
            tc.tile_pool(name="dinv", bufs=2) as dinv_pool,
            tc.tile_pool(name="ostage", bufs=3) as ostage,
            tc.tile_pool(name="ps", bufs=1, space="PSUM") as ps,
        ):
            # ---- SBUF destination tiles ----
            xT_sb = consts.tile([128, KC, S], BF)
            wqh0_sb = consts.tile([128, KC, D], BF)
            wqr_sb = consts.tile([128, KC, (HLOC - 1) * D], BF)
            wk_sb = consts.tile([128, KC, GLOC * D], BF)
            wv_sb = consts.tile([128, KC, GLOC * D], BF)
            wo_sb = consts.tile([128, HLOC, DIM], BF)
            swp_sb = consts.tile([128, D], BF)
            cos_sb = consts.tile([128, S], BF)
            sin_sb = consts.tile([128, S], BF)
            tri_sb = consts.tile([128, D], F32)
            wk_flat = wk_sb.rearrange("p a b -> p (a b)")
            wv_flat = wv_sb.rearrange("p a b -> p (a b)")
            wqh0_flat = wqh0_sb.rearrange("p a b -> p (a b)")

            # ---- DMA dispatches ----
            # SP queue: xT fine-grained at the front (the kc-outer pass is
            # paced by chunk arrival), then wq heads 1-7, then wo.
            for kc in range(4):
                nc.sync.dma_start(out=xT_sb[:, kc, :], in_=xT_r[:, kc, :])
            nc.sync.dma_start(out=xT_sb[:, 4:8, :], in_=xT_r[:, 4:8, :])
            nc.sync.dma_start(out=xT_sb[:, 8:12, :], in_=xT_r[:, 8:12, :])
            nc.sync.dma_start(out=xT_sb[:, 12:16, :], in_=xT_r[:, 12:16, :])
            for g in range(4):
                nc.sync.dma_start(
                    out=wqr_sb[:, 4 * g:4 * g + 4, :], in_=wqr_r[:, 4 * g:4 * g + 4, :]
                )
            nc.sync.dma_start(out=wo_sb[:, 0:4, :], in_=wo_r[:, 0:4, :])
            nc.sync.dma_start(out=wo_sb[:, 4:8, :], in_=wo_r[:, 4:8, :])
            # ACT queue: wk/wq-h0 chunk groups first (pass startup), then the
            # rope constants, then wv (first needed once xT is complete).
            nc.scalar.dma_start(out=wk_flat[:, 0:1024], in_=wkf[:, 0:1024])
            nc.scalar.dma_start(out=wqh0_flat[:, 0:1024], in_=wqh0[:, 0:1024])
            nc.scalar.dma_start(out=wk_flat[:, 1024:4096], in_=wkf[:, 1024:4096])
            nc.scalar.dma_start(out=wqh0_flat[:, 1024:2048], in_=wqh0[:, 1024:2048])
            nc.scalar.dma_start(out=swp_sb, in_=swp[:, :])
            nc.scalar.dma_start(out=cos_sb, in_=cosT[:, :])
            nc.scalar.dma_start(out=sin_sb, in_=sinT[:, :])
            nc.scalar.dma_start(out=wv_flat, in_=wvf[:, :])
            nc.scalar.dma_start(out=tri_sb, in_=tri[:, :])

            # all-ones stationary operand: the denom matmul then writes the
            # key-sum to EVERY output partition (free partition-broadcast)
            ones_sq = consts.tile([128, 128], BF)
            nc.vector.memset(ones_sq, 1.0)

            # persistent activations
            qT_sb = consts.tile([128, HLOC, S], BF)   # q^T, rotated
            kT_sb = consts.tile([128, GLOC, S], BF)   # k^T, rotated
            v_sb = consts.tile([128, NKB, GLOC * D], BF)  # v in [tok, d]
            aT_sb = consts.tile([128, HLOC, S], BF)   # attention out^T

            def bank(i, name):
                return ps.tile([128, 512], F32, tag=f"b{i}", name=name)

            def rope_finish(psrc, tb, dst_sb, dst_idx, ps2_bank):
                """dst[:, dst_idx, tb*512:+512] = rope(psrc) (psrc = w^T@x^T)."""
                qsb = rope_pool.tile([128, 512], BF, tag="qsb")
                nc.scalar.copy(out=qsb, in_=psrc)
                ps2 = bank(ps2_bank, "ps2")
                nc.tensor.matmul(ps2, swp_sb, qsb, start=True, stop=True)
                tspan = slice(tb * 512, (tb + 1) * 512)
                t1 = rope_pool.tile([128, 512], F32, tag="t1")
                nc.vector.tensor_mul(out=t1, in0=qsb, in1=cos_sb[:, tspan])
                t2 = rope_pool.tile([128, 512], F32, tag="t2")
                nc.vector.tensor_mul(out=t2, in0=ps2, in1=sin_sb[:, tspan])
                nc.vector.tensor_add(
                    out=dst_sb[:, dst_idx, tspan], in0=t1, in1=t2
                )

            # ---- phase 1: joint kc-outer K + Q(head0) projection ----
            # All 6 accumulators live across the contraction; ~1.3us of
            # matmul per chunk matches the chunk arrival rate.
            kps = [bank(i, f"kps{i}") for i in range(4)]
            qps = [bank(4, "qps0"), bank(5, "qps1")]
            for kc in range(KC):
                for g in range(GLOC):
                    for tb in range(NTB):
                        nc.tensor.matmul(
                            kps[2 * g + tb],
                            wk_sb[:, kc, g * 128:(g + 1) * 128],
                            xT_sb[:, kc, tb * 512:(tb + 1) * 512],
                            start=(kc == 0),
                            stop=(kc == KC - 1),
                        )
                for tb in range(NTB):
                    nc.tensor.matmul(
                        qps[tb],
                        wqh0_sb[:, kc, :],
                        xT_sb[:, kc, tb * 512:(tb + 1) * 512],
                        start=(kc == 0),
                        stop=(kc == KC - 1),
                    )
            # ropes: kv-group 0 + q head0 first (attention(0) needs them),
            # then kv-group 1 (first used at h=4); ps2 scratch in bank 6
            rope_finish(kps[0], 0, kT_sb, 0, 6)
            rope_finish(kps[1], 1, kT_sb, 0, 6)
            rope_finish(qps[0], 0, qT_sb, 0, 6)
            rope_finish(qps[1], 1, qT_sb, 0, 6)
            rope_finish(kps[2], 0, kT_sb, 1, 6)
            rope_finish(kps[3], 1, kT_sb, 1, 6)

            # ---- v projection (vtb-outer, after xT complete) ----
            for vtb in range(NKB):
                ps_v = bank(6 + vtb % 2, f"vps{vtb}")
                for kc in range(KC):
                    nc.tensor.matmul(
                        ps_v[:, : GLOC * D],
                        xT_sb[:, kc, vtb * 128:(vtb + 1) * 128],
                        wv_sb[:, kc, :],
                        start=(kc == 0),
                        stop=(kc == KC - 1),
                    )
                nc.scalar.copy(out=v_sb[:, vtb, :], in_=ps_v[:, : GLOC * D])

            def proj_rope(h, tb):
                """project q head h (h>=1) for token block tb + rope."""
                pq = bank(6 + tb, f"qp{h}_{tb}")
                for kc in range(KC):
                    nc.tensor.matmul(
                        pq,
                        wqr_sb[:, kc, (h - 1) * 128:h * 128],
                        xT_sb[:, kc, tb * 512:(tb + 1) * 512],
                        start=(kc == 0),
                        stop=(kc == KC - 1),
                    )
                rope_finish(pq, tb, qT_sb, h, 4)

            def attention(h):
                g = h // REP
                for rg in range(NTB):
                    rows = slice(rg * 512, (rg + 1) * 512)
                    pt = pt_pool.tile([128, NKB, 512], BF, tag="pt")
                    po = bank(5 - 3 * rg, f"po{h}_{rg}")  # b5 / b2
                    nkb = 4 * rg + 4

                    def span_of(kb):
                        return slice(max(0, kb - 4 * rg) * 128, 512)

                    def score_exp(kb):
                        c0 = max(0, kb - 4 * rg)
                        span = span_of(kb)
                        psc = bank(kb % 2, f"sc{h}_{rg}_{kb}")  # b0 / b1
                        nc.tensor.matmul(
                            psc[:, span],
                            kT_sb[:, g, kb * 128:(kb + 1) * 128],
                            qT_sb[:, h, rg * 512 + c0 * 128:(rg + 1) * 512],
                            start=True,
                            stop=True,
                        )
                        if kb - 4 * rg >= 0:
                            cc = kb - 4 * rg
                            nc.vector.tensor_add(
                                out=psc[:, cc * 128:(cc + 1) * 128],
                                in0=psc[:, cc * 128:(cc + 1) * 128],
                                in1=tri_sb,
                            )
                        nc.scalar.activation(
                            out=pt[:, kb, span],
                            in_=psc[:, span],
                            func=mybir.ActivationFunctionType.Exp,
                            scale=SCALE,
                        )

                    # scores issued 2 key-blocks ahead so exp(kb) is done
                    # before po(kb) issues
                    acc = acc_pool.tile([128, 512], BF, tag="acc")
                    score_exp(0)
                    if nkb > 1:
                        score_exp(1)
                    for kb in range(nkb):
                        if kb + 2 < nkb:
                            score_exp(kb + 2)
                        span = span_of(kb)
                        nc.tensor.matmul(
                            po[:, span],
                            v_sb[:, kb, g * D:(g + 1) * D],
                            pt[:, kb, span],
                            start=(kb == 0),
                            stop=(kb == nkb - 1),
                        )
                        # denominator: collapse the key-block dim on DVE over
                        # the causal extent only (bf16, 2x DVE rate)
                        if kb == 0:
                            nc.vector.tensor_copy(out=acc, in_=pt[:, 0, :])
                        else:
                            nc.vector.tensor_add(
                                out=acc[:, span], in0=acc[:, span],
                                in1=pt[:, kb, span],
                            )
                    # single ones-matmul: partition-broadcast key sum
                    pd = bank(3, f"pd{h}_{rg}")
                    nc.tensor.matmul(pd, ones_sq, acc, start=True, stop=True)
                    # 1/denom as exp(-ln(denom)) on ACT: DVE reciprocal costs
                    # ~6.5ns/elem (3.3us/tile) and was stalling the PE
                    lnd = dinv_pool.tile([128, 512], F32, tag="lnd")
                    nc.scalar.activation(
                        out=lnd, in_=pd, func=mybir.ActivationFunctionType.Ln
                    )
                    dinv_b = dinv_pool.tile([128, 512], F32, tag="dinvb")
                    nc.scalar.activation(
                        out=dinv_b, in_=lnd,
                        func=mybir.ActivationFunctionType.Exp, scale=-1.0,
                    )
                    nc.vector.tensor_mul(
                        out=aT_sb[:, h, rows],
                        in0=po,
                        in1=dinv_b,
                    )

            # ---- per-head: attend head h, project q for head h+1 ----
            for h in range(HLOC):
                attention(h)
                if h + 1 < HLOC:
                    for tb in range(NTB):
                        proj_rope(h + 1, tb)

            # ---- output projection: out^T = wo_sh^T @ a^T ----
            # tb-outer so the tb=0 tiles (whose aT rows complete first) go
            # first (GPSIMD cannot read PSUM on this target, so DVE evacuates)
            for tb in range(NTB):
                for et in range(KC):
                    pso = bank([0, 1, 5][et % 3], f"op{tb}_{et}")
                    for c in range(HLOC):
                        nc.tensor.matmul(
                            pso,
                            wo_sb[:, c, et * 128:(et + 1) * 128],
                            aT_sb[:, c, tb * 512:(tb + 1) * 512],
                            start=(c == 0),
                            stop=(c == HLOC - 1),
                        )
                    st = ostage.tile([128, 512], BF, tag="st")
                    nc.vector.tensor_copy(out=st, in_=pso)
                    nc.sync.dma_start(
                        out=outT_r[:, et, tb * 512:(tb + 1) * 512], in_=st
                    )

    return nc


LAST_RESULT = None
_TRACE = os.environ.get("BASS_ATTN_TRACE", "") == "1"

if _TRACE:
    # Register the NTFF profile hook that the agent image's antenv lacks
    # (test/profiling only; the graded path never enters this branch).
    try:
        import sys
        import types

        import antenv  # noqa: F401

        if "antenv.axon_hooks" not in sys.modules:
            _mod = types.ModuleType("antenv.axon_hooks")
            _hook_box = [None]
            _mod.set_axon_ntff_profile_hook = lambda h: _hook_box.__setitem__(0, h)
            _mod.get_axon_ntff_profile_hook = lambda: _hook_box[0]
            sys.modules["antenv.axon_hooks"] = _mod
            import antenv as _antenv

            _antenv.axon_hooks = _mod
            from trn_agent_boot.trn_boot import _ntff_profile_via_ctypes

            _mod.set_axon_ntff_profile_hook(
                _ntff_profile_via_ctypes("/opt/axon/libaxon_pjrt.so")
            )
    except Exception as e:  # pragma: no cover
        print(f"NTFF hook setup failed ({e}); tracing will be skipped")


def kernel(x, freqs_cis, wq, wk, wv, wo, seq_len=None, **_ignored):
    global _PROGRAM, LAST_RESULT
    x = np.ascontiguousarray(np.asarray(x, dtype=np.float32))
    fc = np.asarray(freqs_cis, dtype=np.float32)
    wq = np.asarray(wq, dtype=np.float32)
    wk = np.asarray(wk, dtype=np.float32)
    wv = np.asarray(wv, dtype=np.float32)
    wo = np.asarray(wo, dtype=np.float32)

    # host-side prep (sharding + transposed/bf16 views + rope/mask constants)
    xT = np.ascontiguousarray(x.T).astype(BF16)                    # [2048, 4096]
    cos = np.ascontiguousarray(np.repeat(fc[:S, :, 0], 2, axis=1).T).astype(BF16)
    sgn = np.where(np.arange(D) % 2 == 0, -1.0, 1.0).astype(np.float32)
    sin = np.ascontiguousarray(
        (np.repeat(fc[:S, :, 1], 2, axis=1) * sgn[None, :]).T
    ).astype(BF16)
    swp = np.zeros((D, D), BF16)
    swp[np.arange(D), np.arange(D) ^ 1] = 1
    k_idx = np.arange(128)[:, None]
    r_idx = np.arange(128)[None, :]
    tri = np.where(r_idx >= k_idx, 0.0, -1e9).astype(np.float32)

    def pflat(w):
        # partition-major flat layout: out[p, kc*F+f] = w[kc*128+p, f]
        return np.ascontiguousarray(
            w.reshape(16, 128, -1).transpose(1, 0, 2).reshape(128, -1)
        )

    in_maps = []
    for c in range(NCORE):
        s, h2 = c // 2, c % 2
        wq_sh = wq[:, h2 * HLOC * D:(h2 + 1) * HLOC * D].astype(BF16)
        in_maps.append(
            {
                "xT": np.ascontiguousarray(xT[:, s * S:(s + 1) * S]),
                "wqh0": pflat(wq_sh[:, 0:D]),
                "wqr": np.ascontiguousarray(wq_sh[:, D:]),
                "wkf": pflat(wk[:, h2 * GLOC * D:(h2 + 1) * GLOC * D].astype(BF16)),
                "wvf": pflat(wv[:, h2 * GLOC * D:(h2 + 1) * GLOC * D].astype(BF16)),
                "wo": wo[h2 * HLOC * D:(h2 + 1) * HLOC * D, :].astype(BF16),
                "cosT": cos,
                "sinT": sin,
                "swp": swp,
                "tri": tri,
            }
        )

    if _PROGRAM is None:
        _PROGRAM = _build_program()

    res = run_bass_kernel_spmd(
        _PROGRAM, in_maps, core_ids=list(range(NCORE)), trace=_TRACE
    )
    LAST_RESULT = res

    out = np.empty((B * S, DIM), np.float32)
    for s in range(B):
        outT = (
            res.results[2 * s]["outT"].astype(np.float32)
            + res.results[2 * s + 1]["outT"].astype(np.float32)
        )
        out[s * S:(s + 1) * S, :] = outT.T
    return out


# revision 7
# speedup vs baseline: 1.1492x; 1.0127x over previous
"""Trainium2 Bass kernel for packed-segment causal GQA attention.

Shapes (hardcoded): x [4096, 2048], 16 q heads / 4 kv heads, head dim 128,
4 packed segments of 1024 tokens.

Sharding over 8 cores: core c -> segment c//2 (tokens), head-half c%2
(8 q heads + their 2 kv heads; wq/wk/wv column-sharded, wo row-sharded).
Each core computes a partial out^T [2048, 1024] for its segment; host sums
the two partials per segment (wo row-shard) and transposes back.

On-device dataflow (all in transposed token space, bf16 matmuls, fp32 PSUM):
  joint kc-outer pass: k^T (4 tiles) + q0^T (2 tiles) accumulate across the
    16 contraction chunks as the xT chunks stream in, so the PE tracks the
    DMA wave instead of waiting for full tensors
  RoPE via pair-swap matmul + cos/sin elementwise combine
  v in [token, d] layout (vtb-outer after xT completes)
  s^T[key, row] = k^T_tile^T.T @ q^T  per 128-key x 512-row block
  p^T = exp(s^T/sqrt(d)) with additive triangular mask on diagonal chunks
  denom: key-block dim collapsed on DVE (bf16, causal extents only), then
    ONE ones-matmul per row group broadcasts the key-sum to all partitions
    (~8.2K PE cycles vs 36.9K for per-block ones-matmul accumulation)
  o^T += v_tile.T @ p^T;  a^T = o^T * (1/denom) with 1/denom = exp(-ln(d))
  out^T = wo_sh^T @ a^T, evacuated PSUM->SBUF on GPSIMD, DMA out on SP

DMA plan: SP carries xT (fine-grained chunks first) then wq[h1..7] then wo;
ACT carries wk/wq-head0 flats (chunk-grouped, so the kc-outer pass starts as
soon as chunk group 0 lands) then rope constants then wv. Dispatches are
batched into multi-chunk APs to amortize the ~650ns sequencer cost each.
"""

import os
import re

import numpy as np
import ml_dtypes

import bass_rust
import concourse.bass as bass
import concourse.mybir as mybir
import concourse.tile as tile
from concourse.bass_utils import run_bass_kernel_spmd
from concourse.vector_clock import ScopedClock

BF16 = ml_dtypes.bfloat16
F32 = mybir.dt.float32
BF = mybir.dt.bfloat16

DIM, H, HKV, D, B, S = 2048, 16, 4, 128, 4, 1024
REP = H // HKV
SCALE = float(D) ** -0.5
NCORE = 8
HLOC = 8          # q heads per core
GLOC = 2          # kv heads per core


_MAXW = 1


def _patch_wait_split(tilemod):
    """walrus in this env caps sem waits per instruction: rewrite any
    instruction carrying more than _MAXW waits so the excess waits land on
    same-engine NoOps inserted just before it."""

    orig_lower = tilemod.TileContext._lower_ordered_insts

    def _split_block(nc, insts):
        out = []
        for inst in insts:
            si = getattr(inst, "sync_info", None)
            waits = list(si.on_wait) if si is not None and si.on_wait else []
            if len(waits) > _MAXW:
                head, rest = waits[:-_MAXW], waits[-_MAXW:]
                for w in head:  # NoOp is CTRL-class: keep it to 1 wait each
                    out.append(
                        mybir.InstNoOp(
                            name=nc.get_next_instruction_name(),
                            engine=inst.engine,
                            bass_nofuse=True,
                            debug=inst.debug,
                            sync_info=mybir.SyncInfo(on_wait=[w], on_update=[]),
                        )
                    )
                inst.sync_info = mybir.SyncInfo(
                    on_wait=rest, on_update=list(si.on_update)
                )
            out.append(inst)
        insts[:] = out

    def patched(self, ordered):
        for insts in ordered.values():
            _split_block(self.nc, insts)
        return orig_lower(self, ordered)

    tilemod.TileContext._lower_ordered_insts = patched


def _patch_drain(tilemod):
    """walrus in this env rejects >1 sem wait on CTRL instructions: split the
    TileContext-exit drain's waits across single-wait SP NoOPs."""

    def _drain_and_barrier_split(self, tick_clock, wait_clock):
        nc = self.nc
        gc = tick_clock.global_clock
        ticks = [int(t) for t in re.findall(r"\d+", str(gc))]
        for idx, tick in enumerate(ticks):
            if tick <= 0:
                continue
            part = bass_rust.VectorClock()
            part.require_at_least(idx, tick)
            n = nc.sync.nop(hint="drain_split", nofuse=True)
            wait_clock.add_sem_waits(n.ins, ScopedClock({None: part}))
        d = nc.sync.drain()
        wait_clock.add_sem_waits(
            d.ins, ScopedClock({None: gc}), cur_clock=ScopedClock({None: gc})
        )
        nc.all_engine_barrier()
        assert self.sems is not None
        popped = nc._tile_sem_poison_stack.pop()
        assert popped is self._sem_poison
        nc.clear_and_free_semaphores(list(self.sems.allocated().values()))
        nc.all_engine_barrier()

    tilemod.TileContext._drain_and_barrier = _drain_and_barrier_split


_patch_wait_split(tile)
_patch_drain(tile)

_PROGRAM = None


def _build_program():
    nc = bass.Bass()

    xT = nc.declare_dram_parameter("xT", [DIM, S], BF, isOutput=False)
    # head-0 q weights, partition-major flat [128, 16*128] (chunk kc at cols
    # kc*128) so the h0 projection can join the kc-outer pass; heads 1-7 in
    # the natural column layout (chunk DMAs there have 1792B lines)
    wqh0 = nc.declare_dram_parameter("wqh0", [128, (DIM // 128) * D], BF, isOutput=False)
    wqr = nc.declare_dram_parameter("wqr", [DIM, (HLOC - 1) * D], BF, isOutput=False)
    # wk/wv host-rearranged to partition-major [128, KC*256]: big DMA lines
    # run at full queue rate; the natural [128,kc,256] layout has 512B lines
    # at ~40% of that, which starved the projections
    wkf = nc.declare_dram_parameter("wkf", [128, (DIM // 128) * GLOC * D], BF, isOutput=False)
    wvf = nc.declare_dram_parameter("wvf", [128, (DIM // 128) * GLOC * D], BF, isOutput=False)
    wo = nc.declare_dram_parameter("wo", [HLOC * D, DIM], BF, isOutput=False)
    cosT = nc.declare_dram_parameter("cosT", [D, S], BF, isOutput=False)
    sinT = nc.declare_dram_parameter("sinT", [D, S], BF, isOutput=False)
    swp = nc.declare_dram_parameter("swp", [D, D], BF, isOutput=False)
    tri = nc.declare_dram_parameter("tri", [D, D], F32, isOutput=False)
    outT = nc.declare_dram_parameter("outT", [DIM, S], BF, isOutput=True)

    xT_r = xT.rearrange("(o p) t -> p o t", p=128)      # [128, 16, 1024]
    wqr_r = wqr.rearrange("(o p) f -> p o f", p=128)    # [128, 16, 896]
    wo_r = wo.rearrange("(c p) e -> p c e", p=128)      # [128, 8, 2048]
    outT_r = outT.rearrange("(o p) t -> p o t", p=128)  # [128, 16, 1024]

    KC = DIM // 128      # 16 contraction chunks
    NTB = S // 512       # 2 token blocks of 512
    NKB = S // 128       # 8 key blocks of 128

    with tile.TileContext(nc) as tc:
        with (
            tc.tile_pool(name="consts", bufs=1) as consts,
            tc.tile_pool(name="rope", bufs=2) as rope_pool,
            tc.tile_pool(name="pt", bufs=2) as pt_pool,
            tc.tile_pool(name="acc", bufs=2) as acc_pool,
            tc.tile_pool(name="dinv", bufs=2) as dinv_pool,
            tc.tile_pool(name="ostage", bufs=3) as ostage,
            tc.tile_pool(name="ps", bufs=1, space="PSUM") as ps,
        ):
            # ---- SBUF destination tiles ----
            xT_sb = consts.tile([128, KC, S], BF)
            wqh0_sb = consts.tile([128, KC, D], BF)
            wqr_sb = consts.tile([128, KC, (HLOC - 1) * D], BF)
            wk_sb = consts.tile([128, KC, GLOC * D], BF)
            wv_sb = consts.tile([128, KC, GLOC * D], BF)
            wo_sb = consts.tile([128, HLOC, DIM], BF)
            swp_sb = consts.tile([128, D], BF)
            cos_sb = consts.tile([128, S], BF)
            sin_sb = consts.tile([128, S], BF)
            tri_sb = consts.tile([128, D], F32)
            wk_flat = wk_sb.rearrange("p a b -> p (a b)")
            wv_flat = wv_sb.rearrange("p a b -> p (a b)")
            wqh0_flat = wqh0_sb.rearrange("p a b -> p (a b)")

            # ---- DMA dispatches ----
            # SP queue: xT fine-grained at the front (the kc-outer pass is
            # paced by chunk arrival), then wq heads 1-7, then wo. DMA
            # engines round-robin over outstanding descriptors, so the very
            # first dependencies are kept small (chunk 0 split by token
            # block) to complete early.
            nc.sync.dma_start(out=xT_sb[:, 0, 0:512], in_=xT_r[:, 0, 0:512])
            nc.sync.dma_start(out=xT_sb[:, 0, 512:1024], in_=xT_r[:, 0, 512:1024])
            for kc in range(1, 4):
                nc.sync.dma_start(out=xT_sb[:, kc, :], in_=xT_r[:, kc, :])
            nc.sync.dma_start(out=xT_sb[:, 4:8, :], in_=xT_r[:, 4:8, :])
            nc.sync.dma_start(out=xT_sb[:, 8:12, :], in_=xT_r[:, 8:12, :])
            nc.sync.dma_start(out=xT_sb[:, 12:16, :], in_=xT_r[:, 12:16, :])
            for g in range(4):
                nc.sync.dma_start(
                    out=wqr_sb[:, 4 * g:4 * g + 4, :], in_=wqr_r[:, 4 * g:4 * g + 4, :]
                )
            nc.sync.dma_start(out=wo_sb[:, 0:4, :], in_=wo_r[:, 0:4, :])
            nc.sync.dma_start(out=wo_sb[:, 4:8, :], in_=wo_r[:, 4:8, :])
            # ACT queue: wk/wq-h0 chunk groups first (pass startup), then the
            # rope constants, then wv (first needed once xT is complete).
            nc.scalar.dma_start(out=wk_flat[:, 0:512], in_=wkf[:, 0:512])
            nc.scalar.dma_start(out=wqh0_flat[:, 0:256], in_=wqh0[:, 0:256])
            nc.scalar.dma_start(out=wk_flat[:, 512:1024], in_=wkf[:, 512:1024])
            nc.scalar.dma_start(out=wqh0_flat[:, 256:1024], in_=wqh0[:, 256:1024])
            nc.scalar.dma_start(out=wk_flat[:, 1024:4096], in_=wkf[:, 1024:4096])
            nc.scalar.dma_start(out=wqh0_flat[:, 1024:2048], in_=wqh0[:, 1024:2048])
            nc.scalar.dma_start(out=swp_sb, in_=swp[:, :])
            nc.scalar.dma_start(out=cos_sb, in_=cosT[:, :])
            nc.scalar.dma_start(out=sin_sb, in_=sinT[:, :])
            nc.scalar.dma_start(out=wv_flat, in_=wvf[:, :])
            nc.scalar.dma_start(out=tri_sb, in_=tri[:, :])

            # all-ones stationary operand: the denom matmul then writes the
            # key-sum to EVERY output partition (free partition-broadcast)
            ones_sq = consts.tile([128, 128], BF)
            nc.vector.memset(ones_sq, 1.0)

            # persistent activations
            qT_sb = consts.tile([128, HLOC, S], BF)   # q^T, rotated
            kT_sb = consts.tile([128, GLOC, S], BF)   # k^T, rotated
            v_sb = consts.tile([128, NKB, GLOC * D], BF)  # v in [tok, d]
            aT_sb = consts.tile([128, HLOC, S], BF)   # attention out^T

            def bank(i, name):
                return ps.tile([128, 512], F32, tag=f"b{i}", name=name)

            def rope_finish(psrc, tb, dst_sb, dst_idx, ps2_bank):
                """dst[:, dst_idx, tb*512:+512] = rope(psrc) (psrc = w^T@x^T)."""
                qsb = rope_pool.tile([128, 512], BF, tag="qsb")
                nc.scalar.copy(out=qsb, in_=psrc)
                ps2 = bank(ps2_bank, "ps2")
                nc.tensor.matmul(ps2, swp_sb, qsb, start=True, stop=True)
                tspan = slice(tb * 512, (tb + 1) * 512)
                t1 = rope_pool.tile([128, 512], F32, tag="t1")
                nc.vector.tensor_mul(out=t1, in0=qsb, in1=cos_sb[:, tspan])
                t2 = rope_pool.tile([128, 512], F32, tag="t2")
                nc.vector.tensor_mul(out=t2, in0=ps2, in1=sin_sb[:, tspan])
                nc.vector.tensor_add(
                    out=dst_sb[:, dst_idx, tspan], in0=t1, in1=t2
                )

            # ---- phase 1: joint kc-outer K + Q(head0) projection ----
            # All 6 accumulators live across the contraction; ~1.3us of
            # matmul per chunk matches the chunk arrival rate.
            kps = [bank(i, f"kps{i}") for i in range(4)]
            qps = [bank(4, "qps0"), bank(5, "qps1")]
            for kc in range(KC):
                for g in range(GLOC):
                    for tb in range(NTB):
                        nc.tensor.matmul(
                            kps[2 * g + tb],
                            wk_sb[:, kc, g * 128:(g + 1) * 128],
                            xT_sb[:, kc, tb * 512:(tb + 1) * 512],
                            start=(kc == 0),
                            stop=(kc == KC - 1),
                        )
                for tb in range(NTB):
                    nc.tensor.matmul(
                        qps[tb],
                        wqh0_sb[:, kc, :],
                        xT_sb[:, kc, tb * 512:(tb + 1) * 512],
                        start=(kc == 0),
                        stop=(kc == KC - 1),
                    )
            # ropes: kv-group 0 + q head0 first (attention(0) needs them),
            # then kv-group 1 (first used at h=4); ps2 scratch in bank 6
            rope_finish(kps[0], 0, kT_sb, 0, 6)
            rope_finish(kps[1], 1, kT_sb, 0, 6)
            rope_finish(qps[0], 0, qT_sb, 0, 6)
            rope_finish(qps[1], 1, qT_sb, 0, 6)
            rope_finish(kps[2], 0, kT_sb, 1, 6)
            rope_finish(kps[3], 1, kT_sb, 1, 6)

            # ---- v projection (vtb-outer, after xT complete) ----
            for vtb in range(NKB):
                ps_v = bank(6 + vtb % 2, f"vps{vtb}")
                for kc in range(KC):
                    nc.tensor.matmul(
                        ps_v[:, : GLOC * D],
                        xT_sb[:, kc, vtb * 128:(vtb + 1) * 128],
                        wv_sb[:, kc, :],
                        start=(kc == 0),
                        stop=(kc == KC - 1),
                    )
                nc.scalar.copy(out=v_sb[:, vtb, :], in_=ps_v[:, : GLOC * D])

            def proj_rope(h, tb):
                """project q head h (h>=1) for token block tb + rope."""
                pq = bank(6 + tb, f"qp{h}_{tb}")
                for kc in range(KC):
                    nc.tensor.matmul(
                        pq,
                        wqr_sb[:, kc, (h - 1) * 128:h * 128],
                        xT_sb[:, kc, tb * 512:(tb + 1) * 512],
                        start=(kc == 0),
                        stop=(kc == KC - 1),
                    )
                rope_finish(pq, tb, qT_sb, h, 4)

            def attention(h):
                g = h // REP
                for rg in range(NTB):
                    rows = slice(rg * 512, (rg + 1) * 512)
                    pt = pt_pool.tile([128, NKB, 512], BF, tag="pt")
                    po = bank(5 - 3 * rg, f"po{h}_{rg}")  # b5 / b2
                    nkb = 4 * rg + 4

                    def span_of(kb):
                        return slice(max(0, kb - 4 * rg) * 128, 512)

                    def score_exp(kb):
                        c0 = max(0, kb - 4 * rg)
                        span = span_of(kb)
                        psc = bank(kb % 2, f"sc{h}_{rg}_{kb}")  # b0 / b1
                        nc.tensor.matmul(
                            psc[:, span],
                            kT_sb[:, g, kb * 128:(kb + 1) * 128],
                            qT_sb[:, h, rg * 512 + c0 * 128:(rg + 1) * 512],
                            start=True,
                            stop=True,
                        )
                        if kb - 4 * rg >= 0:
                            cc = kb - 4 * rg
                            nc.vector.tensor_add(
                                out=psc[:, cc * 128:(cc + 1) * 128],
                                in0=psc[:, cc * 128:(cc + 1) * 128],
                                in1=tri_sb,
                            )
                        nc.scalar.activation(
                            out=pt[:, kb, span],
                            in_=psc[:, span],
                            func=mybir.ActivationFunctionType.Exp,
                            scale=SCALE,
                        )

                    # scores issued 2 key-blocks ahead so exp(kb) is done
                    # before po(kb) issues
                    acc = acc_pool.tile([128, 512], BF, tag="acc")
                    score_exp(0)
                    if nkb > 1:
                        score_exp(1)
                    for kb in range(nkb):
                        if kb + 2 < nkb:
                            score_exp(kb + 2)
                        span = span_of(kb)
                        nc.tensor.matmul(
                            po[:, span],
                            v_sb[:, kb, g * D:(g + 1) * D],
                            pt[:, kb, span],
                            start=(kb == 0),
                            stop=(kb == nkb - 1),
                        )
                        # denominator: collapse the key-block dim on DVE over
                        # the causal extent only (bf16, 2x DVE rate)
                        if kb == 0:
                            nc.vector.tensor_copy(out=acc, in_=pt[:, 0, :])
                        else:
                            nc.vector.tensor_add(
                                out=acc[:, span], in0=acc[:, span],
                                in1=pt[:, kb, span],
                            )
                    # single ones-matmul: partition-broadcast key sum
                    pd = bank(3, f"pd{h}_{rg}")
                    nc.tensor.matmul(pd, ones_sq, acc, start=True, stop=True)
                    # 1/denom as exp(-ln(denom)) on ACT: DVE reciprocal costs
                    # ~6.5ns/elem (3.3us/tile) and was stalling the PE
                    lnd = dinv_pool.tile([128, 512], F32, tag="lnd")
                    nc.scalar.activation(
                        out=lnd, in_=pd, func=mybir.ActivationFunctionType.Ln
                    )
                    dinv_b = dinv_pool.tile([128, 512], F32, tag="dinvb")
                    nc.scalar.activation(
                        out=dinv_b, in_=lnd,
                        func=mybir.ActivationFunctionType.Exp, scale=-1.0,
                    )
                    nc.vector.tensor_mul(
                        out=aT_sb[:, h, rows],
                        in0=po,
                        in1=dinv_b,
                    )

            # ---- per-head: attend head h, project q for head h+1 ----
            for h in range(HLOC):
                attention(h)
                if h + 1 < HLOC:
                    for tb in range(NTB):
                        proj_rope(h + 1, tb)

            # ---- output projection: out^T = wo_sh^T @ a^T ----
            # tb-outer so the tb=0 tiles (whose aT rows complete first) go
            # first (GPSIMD cannot read PSUM on this target, so DVE evacuates)
            # banks b4/b6/b7 are idle during the last head's attention
            # (no next-head q projection), so the tb=0 tiles — which only
            # depend on every head's rg=0 rows — fill h7-rg1's exp stalls
            for tb in range(NTB):
                for et in range(KC):
                    pso = bank([4, 6, 7][et % 3], f"op{tb}_{et}")
                    for c in range(HLOC):
                        nc.tensor.matmul(
                            pso,
                            wo_sb[:, c, et * 128:(et + 1) * 128],
                            aT_sb[:, c, tb * 512:(tb + 1) * 512],
                            start=(c == 0),
                            stop=(c == HLOC - 1),
                        )
                    st = ostage.tile([128, 512], BF, tag="st")
                    nc.vector.tensor_copy(out=st, in_=pso)
                    nc.sync.dma_start(
                        out=outT_r[:, et, tb * 512:(tb + 1) * 512], in_=st
                    )

    return nc


LAST_RESULT = None
_TRACE = os.environ.get("BASS_ATTN_TRACE", "") == "1"

if _TRACE:
    # Register the NTFF profile hook that the agent image's antenv lacks
    # (test/profiling only; the graded path never enters this branch).
    try:
        import sys
        import types

        import antenv  # noqa: F401

        if "antenv.axon_hooks" not in sys.modules:
            _mod = types.ModuleType("antenv.axon_hooks")
            _hook_box = [None]
            _mod.set_axon_ntff_profile_hook = lambda h: _hook_box.__setitem__(0, h)
            _mod.get_axon_ntff_profile_hook = lambda: _hook_box[0]
            sys.modules["antenv.axon_hooks"] = _mod
            import antenv as _antenv

            _antenv.axon_hooks = _mod
            from trn_agent_boot.trn_boot import _ntff_profile_via_ctypes

            _mod.set_axon_ntff_profile_hook(
                _ntff_profile_via_ctypes("/opt/axon/libaxon_pjrt.so")
            )
    except Exception as e:  # pragma: no cover
        print(f"NTFF hook setup failed ({e}); tracing will be skipped")


def kernel(x, freqs_cis, wq, wk, wv, wo, seq_len=None, **_ignored):
    global _PROGRAM, LAST_RESULT
    x = np.ascontiguousarray(np.asarray(x, dtype=np.float32))
    fc = np.asarray(freqs_cis, dtype=np.float32)
    wq = np.asarray(wq, dtype=np.float32)
    wk = np.asarray(wk, dtype=np.float32)
    wv = np.asarray(wv, dtype=np.float32)
    wo = np.asarray(wo, dtype=np.float32)

    # host-side prep (sharding + transposed/bf16 views + rope/mask constants)
    xT = np.ascontiguousarray(x.T).astype(BF16)                    # [2048, 4096]
    cos = np.ascontiguousarray(np.repeat(fc[:S, :, 0], 2, axis=1).T).astype(BF16)
    sgn = np.where(np.arange(D) % 2 == 0, -1.0, 1.0).astype(np.float32)
    sin = np.ascontiguousarray(
        (np.repeat(fc[:S, :, 1], 2, axis=1) * sgn[None, :]).T
    ).astype(BF16)
    swp = np.zeros((D, D), BF16)
    swp[np.arange(D), np.arange(D) ^ 1] = 1
    k_idx = np.arange(128)[:, None]
    r_idx = np.arange(128)[None, :]
    tri = np.where(r_idx >= k_idx, 0.0, -1e9).astype(np.float32)

    def pflat(w):
        # partition-major flat layout: out[p, kc*F+f] = w[kc*128+p, f]
        return np.ascontiguousarray(
            w.reshape(16, 128, -1).transpose(1, 0, 2).reshape(128, -1)
        )

    in_maps = []
    for c in range(NCORE):
        s, h2 = c // 2, c % 2
        wq_sh = wq[:, h2 * HLOC * D:(h2 + 1) * HLOC * D].astype(BF16)
        in_maps.append(
            {
                "xT": np.ascontiguousarray(xT[:, s * S:(s + 1) * S]),
                "wqh0": pflat(wq_sh[:, 0:D]),
                "wqr": np.ascontiguousarray(wq_sh[:, D:]),
                "wkf": pflat(wk[:, h2 * GLOC * D:(h2 + 1) * GLOC * D].astype(BF16)),
                "wvf": pflat(wv[:, h2 * GLOC * D:(h2 + 1) * GLOC * D].astype(BF16)),
                "wo": wo[h2 * HLOC * D:(h2 + 1) * HLOC * D, :].astype(BF16),
                "cosT": cos,
                "sinT": sin,
                "swp": swp,
                "tri": tri,
            }
        )

    if _PROGRAM is None:
        _PROGRAM = _build_program()

    res = run_bass_kernel_spmd(
        _PROGRAM, in_maps, core_ids=list(range(NCORE)), trace=_TRACE
    )
    LAST_RESULT = res

    out = np.empty((B * S, DIM), np.float32)
    for s in range(B):
        outT = (
            res.results[2 * s]["outT"].astype(np.float32)
            + res.results[2 * s + 1]["outT"].astype(np.float32)
        )
        out[s * S:(s + 1) * S, :] = outT.T
    return out


# revision 18
# speedup vs baseline: 1.1728x; 1.0205x over previous
"""Trainium2 Bass kernel for packed-segment causal GQA attention.

Shapes (hardcoded): x [4096, 2048], 16 q heads / 4 kv heads, head dim 128,
4 packed segments of 1024 tokens.

Sharding over 8 cores: core c -> segment c//2 (tokens), head-half c%2
(8 q heads + their 2 kv heads; wq/wk/wv column-sharded, wo row-sharded).
Each core computes a partial out^T [2048, 1024] for its segment; host sums
the two partials per segment (wo row-shard) and transposes back.

On-device dataflow (all in transposed token space, bf16 matmuls, fp32 PSUM):
  joint kc-outer pass: k^T (4 tiles) + q0^T (2 tiles) accumulate across the
    16 contraction chunks as the xT chunks stream in, so the PE tracks the
    DMA wave instead of waiting for full tensors
  RoPE via pair-swap matmul + cos/sin elementwise combine
  v in [token, d] layout (vtb-outer after xT completes)
  s^T[key, row] = k^T_tile^T.T @ q^T  per 128-key x 512-row block
  p^T = exp(s^T/sqrt(d)) with additive triangular mask on diagonal chunks
  denom: key-block dim collapsed on DVE (bf16, causal extents only), then
    ONE ones-matmul per row group broadcasts the key-sum to all partitions
    (~8.2K PE cycles vs 36.9K for per-block ones-matmul accumulation)
  o^T += v_tile.T @ p^T;  a^T = o^T * (1/denom) with 1/denom = exp(-ln(d))
  out^T = wo_sh^T @ a^T, evacuated PSUM->SBUF on GPSIMD, DMA out on SP

DMA plan: SP carries xT (fine-grained chunks first) then wq[h1..7] then wo;
ACT carries wk/wq-head0 flats (chunk-grouped, so the kc-outer pass starts as
soon as chunk group 0 lands) then rope constants then wv. Dispatches are
batched into multi-chunk APs to amortize the ~650ns sequencer cost each.
"""

import os
import re

import numpy as np
import ml_dtypes

import bass_rust
import concourse.bass as bass
import concourse.mybir as mybir
import concourse.tile as tile
from concourse.bass_utils import run_bass_kernel_spmd
from concourse.vector_clock import ScopedClock

BF16 = ml_dtypes.bfloat16
F32 = mybir.dt.float32
BF = mybir.dt.bfloat16

DIM, H, HKV, D, B, S = 2048, 16, 4, 128, 4, 1024
REP = H // HKV
SCALE = float(D) ** -0.5
NCORE = 8
HLOC = 8          # q heads per core
GLOC = 2          # kv heads per core


_MAXW = 1


def _patch_wait_split(tilemod):
    """walrus in this env caps sem waits per instruction: rewrite any
    instruction carrying more than _MAXW waits so the excess waits land on
    same-engine NoOps inserted just before it."""

    orig_lower = tilemod.TileContext._lower_ordered_insts

    def _split_block(nc, insts):
        out = []
        for inst in insts:
            si = getattr(inst, "sync_info", None)
            waits = list(si.on_wait) if si is not None and si.on_wait else []
            if len(waits) > _MAXW:
                head, rest = waits[:-_MAXW], waits[-_MAXW:]
                for w in head:  # NoOp is CTRL-class: keep it to 1 wait each
                    out.append(
                        mybir.InstNoOp(
                            name=nc.get_next_instruction_name(),
                            engine=inst.engine,
                            bass_nofuse=True,
                            debug=inst.debug,
                            sync_info=mybir.SyncInfo(on_wait=[w], on_update=[]),
                        )
                    )
                inst.sync_info = mybir.SyncInfo(
                    on_wait=rest, on_update=list(si.on_update)
                )
            out.append(inst)
        insts[:] = out

    def patched(self, ordered):
        for insts in ordered.values():
            _split_block(self.nc, insts)
        return orig_lower(self, ordered)

    tilemod.TileContext._lower_ordered_insts = patched


def _patch_drain(tilemod):
    """walrus in this env rejects >1 sem wait on CTRL instructions: split the
    TileContext-exit drain's waits across single-wait SP NoOPs."""

    def _drain_and_barrier_split(self, tick_clock, wait_clock):
        nc = self.nc
        gc = tick_clock.global_clock
        ticks = [int(t) for t in re.findall(r"\d+", str(gc))]
        for idx, tick in enumerate(ticks):
            if tick <= 0:
                continue
            part = bass_rust.VectorClock()
            part.require_at_least(idx, tick)
            n = nc.sync.nop(hint="drain_split", nofuse=True)
            wait_clock.add_sem_waits(n.ins, ScopedClock({None: part}))
        d = nc.sync.drain()
        wait_clock.add_sem_waits(
            d.ins, ScopedClock({None: gc}), cur_clock=ScopedClock({None: gc})
        )
        nc.all_engine_barrier()
        assert self.sems is not None
        popped = nc._tile_sem_poison_stack.pop()
        assert popped is self._sem_poison
        nc.clear_and_free_semaphores(list(self.sems.allocated().values()))
        nc.all_engine_barrier()

    tilemod.TileContext._drain_and_barrier = _drain_and_barrier_split


_patch_wait_split(tile)
_patch_drain(tile)

_PROGRAM = None


def _build_program():
    nc = bass.Bass()

    xT = nc.declare_dram_parameter("xT", [DIM, S], BF, isOutput=False)
    # head-0 q weights, partition-major flat [128, 16*128] (chunk kc at cols
    # kc*128) so the h0 projection can join the kc-outer pass; heads 1-7 in
    # the natural column layout (chunk DMAs there have 1792B lines)
    wqh0 = nc.declare_dram_parameter("wqh0", [128, (DIM // 128) * D], BF, isOutput=False)
    wqr = nc.declare_dram_parameter("wqr", [DIM, (HLOC - 1) * D], BF, isOutput=False)
    # wk/wv host-rearranged to partition-major [128, KC*256]: big DMA lines
    # run at full queue rate; the natural [128,kc,256] layout has 512B lines
    # at ~40% of that, which starved the projections
    wkf = nc.declare_dram_parameter("wkf", [128, (DIM // 128) * GLOC * D], BF, isOutput=False)
    wvf = nc.declare_dram_parameter("wvf", [128, (DIM // 128) * GLOC * D], BF, isOutput=False)
    wo = nc.declare_dram_parameter("wo", [HLOC * D, DIM], BF, isOutput=False)
    cosT = nc.declare_dram_parameter("cosT", [D, S], BF, isOutput=False)
    sinT = nc.declare_dram_parameter("sinT", [D, S], BF, isOutput=False)
    tri = nc.declare_dram_parameter("tri", [D, D], F32, isOutput=False)
    outT = nc.declare_dram_parameter("outT", [DIM, S], BF, isOutput=True)

    xT_r = xT.rearrange("(o p) t -> p o t", p=128)      # [128, 16, 1024]
    wqr_r = wqr.rearrange("(o p) f -> p o f", p=128)    # [128, 16, 896]
    wo_r = wo.rearrange("(c p) e -> p c e", p=128)      # [128, 8, 2048]
    outT_r = outT.rearrange("(o p) t -> p o t", p=128)  # [128, 16, 1024]

    KC = DIM // 128      # 16 contraction chunks
    NTB = S // 512       # 2 token blocks of 512
    NKB = S // 128       # 8 key blocks of 128

    with tile.TileContext(nc) as tc:
        with (
            tc.tile_pool(name="consts", bufs=1) as consts,
            tc.tile_pool(name="rope", bufs=2) as rope_pool,
            tc.tile_pool(name="pt", bufs=2) as pt_pool,
            tc.tile_pool(name="acc", bufs=2) as acc_pool,
            tc.tile_pool(name="dinv", bufs=2) as dinv_pool,
            tc.tile_pool(name="ostage", bufs=3) as ostage,
            tc.tile_pool(name="ps", bufs=1, space="PSUM") as ps,
        ):
            # ---- SBUF destination tiles ----
            xT_sb = consts.tile([128, KC, S], BF)
            wqh0_sb = consts.tile([128, KC, D], BF)
            wqr_sb = consts.tile([128, KC, (HLOC - 1) * D], BF)
            wk_sb = consts.tile([128, KC, GLOC * D], BF)
            wv_sb = consts.tile([128, KC, GLOC * D], BF)
            wo_sb = consts.tile([128, HLOC, DIM], BF)
            cos_sb = consts.tile([128, S], BF)
            sin_sb = consts.tile([128, S], BF)
            tri_sb = consts.tile([128, D], F32)
            wk_flat = wk_sb.rearrange("p a b -> p (a b)")
            wv_flat = wv_sb.rearrange("p a b -> p (a b)")
            wqh0_flat = wqh0_sb.rearrange("p a b -> p (a b)")

            # ---- DMA dispatches ----
            # SP queue: xT fine-grained at the front (the kc-outer pass is
            # paced by chunk arrival), then wq heads 1-7, then wo. DMA
            # engines round-robin over outstanding descriptors, so the very
            # first dependencies are kept small (chunk 0 split by token
            # block) to complete early.
            nc.sync.dma_start(out=xT_sb[:, 0, 0:512], in_=xT_r[:, 0, 0:512])
            nc.sync.dma_start(out=xT_sb[:, 0, 512:1024], in_=xT_r[:, 0, 512:1024])
            for kc in range(1, 4):
                nc.sync.dma_start(out=xT_sb[:, kc, :], in_=xT_r[:, kc, :])
            nc.sync.dma_start(out=xT_sb[:, 4:8, :], in_=xT_r[:, 4:8, :])
            nc.sync.dma_start(out=xT_sb[:, 8:12, :], in_=xT_r[:, 8:12, :])
            nc.sync.dma_start(out=xT_sb[:, 12:16, :], in_=xT_r[:, 12:16, :])
            for g in range(4):
                nc.sync.dma_start(
                    out=wqr_sb[:, 4 * g:4 * g + 4, :], in_=wqr_r[:, 4 * g:4 * g + 4, :]
                )
            nc.sync.dma_start(out=wo_sb[:, 0:4, :], in_=wo_r[:, 0:4, :])
            nc.sync.dma_start(out=wo_sb[:, 4:8, :], in_=wo_r[:, 4:8, :])
            # ACT queue: wk/wq-h0 chunk groups first (pass startup), then the
            # rope constants, then wv (first needed once xT is complete).
            nc.scalar.dma_start(out=wk_flat[:, 0:512], in_=wkf[:, 0:512])
            nc.scalar.dma_start(out=wqh0_flat[:, 0:256], in_=wqh0[:, 0:256])
            nc.scalar.dma_start(out=wk_flat[:, 512:1024], in_=wkf[:, 512:1024])
            nc.scalar.dma_start(out=wqh0_flat[:, 256:1024], in_=wqh0[:, 256:1024])
            nc.scalar.dma_start(out=wk_flat[:, 1024:4096], in_=wkf[:, 1024:4096])
            nc.scalar.dma_start(out=wqh0_flat[:, 1024:2048], in_=wqh0[:, 1024:2048])
            nc.scalar.dma_start(out=cos_sb, in_=cosT[:, :])
            nc.scalar.dma_start(out=sin_sb, in_=sinT[:, :])
            nc.scalar.dma_start(out=wv_flat, in_=wvf[:, :])
            nc.scalar.dma_start(out=tri_sb, in_=tri[:, :])

            # all-ones stationary operand: the denom matmul then writes the
            # key-sum to EVERY output partition (free partition-broadcast)
            ones_sq = consts.tile([128, 128], BF)
            nc.vector.memset(ones_sq, 1.0)

            # persistent activations
            qT_sb = consts.tile([128, HLOC, S], BF)   # q^T, rotated
            kT_sb = consts.tile([128, GLOC, S], BF)   # k^T, rotated
            v_sb = consts.tile([128, NKB, GLOC * D], BF)  # v in [tok, d]
            aT_sb = consts.tile([128, HLOC, S], BF)   # attention out^T

            def bank(i, name):
                return ps.tile([128, 512], F32, tag=f"b{i}", name=name)

            def rope_finish(psrc, tb, dst_sb, dst_idx):
                """dst[:, dst_idx, tb*512:+512] = rope(psrc) (psrc = w^T@x^T).

                Head dims are host-permuted to halves layout (partitions
                0:64 = even pair elements, 64:128 = odd), so the pair swap
                is two partition-base-shifted DVE muls; sin_sb carries the
                sign (-sin on rows 0:64, +sin on rows 64:128)."""
                tspan = slice(tb * 512, (tb + 1) * 512)
                t1 = rope_pool.tile([128, 512], F32, tag="t1")
                nc.vector.tensor_mul(out=t1, in0=psrc, in1=cos_sb[:, tspan])
                t2 = rope_pool.tile([128, 512], F32, tag="t2")
                nc.vector.tensor_mul(
                    out=t2[0:64, :], in0=psrc[64:128, :], in1=sin_sb[0:64, tspan]
                )
                nc.vector.tensor_mul(
                    out=t2[64:128, :], in0=psrc[0:64, :], in1=sin_sb[64:128, tspan]
                )
                nc.vector.tensor_add(
                    out=dst_sb[:, dst_idx, tspan], in0=t1, in1=t2
                )

            # ---- phase 1: joint kc-outer K + Q(head0) projection ----
            # All 6 accumulators live across the contraction; ~1.3us of
            # matmul per chunk matches the chunk arrival rate.
            kps = [bank(i, f"kps{i}") for i in range(4)]
            qps = [bank(4, "qps0"), bank(5, "qps1")]
            for kc in range(KC):
                for g in range(GLOC):
                    for tb in range(NTB):
                        nc.tensor.matmul(
                            kps[2 * g + tb],
                            wk_sb[:, kc, g * 128:(g + 1) * 128],
                            xT_sb[:, kc, tb * 512:(tb + 1) * 512],
                            start=(kc == 0),
                            stop=(kc == KC - 1),
                        )
                for tb in range(NTB):
                    nc.tensor.matmul(
                        qps[tb],
                        wqh0_sb[:, kc, :],
                        xT_sb[:, kc, tb * 512:(tb + 1) * 512],
                        start=(kc == 0),
                        stop=(kc == KC - 1),
                    )
            # ropes: kv-group 0 + q head0 first (attention(0) needs them),
            # then kv-group 1 (first used at h=4)
            rope_finish(kps[0], 0, kT_sb, 0)
            rope_finish(kps[1], 1, kT_sb, 0)
            rope_finish(qps[0], 0, qT_sb, 0)
            rope_finish(qps[1], 1, qT_sb, 0)
            rope_finish(kps[2], 0, kT_sb, 1)
            rope_finish(kps[3], 1, kT_sb, 1)

            # ---- v projection (vtb-outer, after xT complete) ----
            for vtb in range(NKB):
                ps_v = bank(6 + vtb % 2, f"vps{vtb}")
                for kc in range(KC):
                    nc.tensor.matmul(
                        ps_v[:, : GLOC * D],
                        xT_sb[:, kc, vtb * 128:(vtb + 1) * 128],
                        wv_sb[:, kc, :],
                        start=(kc == 0),
                        stop=(kc == KC - 1),
                    )
                nc.scalar.copy(out=v_sb[:, vtb, :], in_=ps_v[:, : GLOC * D])

            def proj_rope(h, tb):
                """project q head h (h>=1) for token block tb + rope."""
                pq = bank(6 + tb, f"qp{h}_{tb}")
                for kc in range(KC):
                    nc.tensor.matmul(
                        pq,
                        wqr_sb[:, kc, (h - 1) * 128:h * 128],
                        xT_sb[:, kc, tb * 512:(tb + 1) * 512],
                        start=(kc == 0),
                        stop=(kc == KC - 1),
                    )
                rope_finish(pq, tb, qT_sb, h)

            def attention(h):
                g = h // REP
                for rg in range(NTB):
                    rows = slice(rg * 512, (rg + 1) * 512)
                    pt = pt_pool.tile([128, NKB, 512], BF, tag="pt")
                    po = bank(5 - 3 * rg, f"po{h}_{rg}")  # b5 / b2
                    nkb = 4 * rg + 4

                    def span_of(kb):
                        return slice(max(0, kb - 4 * rg) * 128, 512)

                    def score_exp(kb):
                        c0 = max(0, kb - 4 * rg)
                        span = span_of(kb)
                        psc = bank([0, 1, 4][kb % 3], f"sc{h}_{rg}_{kb}")
                        nc.tensor.matmul(
                            psc[:, span],
                            kT_sb[:, g, kb * 128:(kb + 1) * 128],
                            qT_sb[:, h, rg * 512 + c0 * 128:(rg + 1) * 512],
                            start=True,
                            stop=True,
                        )
                        if kb - 4 * rg >= 0:
                            cc = kb - 4 * rg
                            nc.vector.tensor_add(
                                out=psc[:, cc * 128:(cc + 1) * 128],
                                in0=psc[:, cc * 128:(cc + 1) * 128],
                                in1=tri_sb,
                            )
                        nc.scalar.activation(
                            out=pt[:, kb, span],
                            in_=psc[:, span],
                            func=mybir.ActivationFunctionType.Exp,
                            scale=SCALE,
                        )

                    # scores issued 3 key-blocks ahead so the ACT exp chain
                    # runs ahead of the po matmuls
                    acc = acc_pool.tile([128, 512], BF, tag="acc")
                    for kb0 in range(min(3, nkb)):
                        score_exp(kb0)
                    for kb in range(nkb):
                        if kb + 3 < nkb:
                            score_exp(kb + 3)
                        span = span_of(kb)
                        nc.tensor.matmul(
                            po[:, span],
                            v_sb[:, kb, g * D:(g + 1) * D],
                            pt[:, kb, span],
                            start=(kb == 0),
                            stop=(kb == nkb - 1),
                        )
                        # denominator: collapse the key-block dim on DVE over
                        # the causal extent only (bf16, 2x DVE rate)
                        if kb == 0:
                            nc.vector.tensor_copy(out=acc, in_=pt[:, 0, :])
                        else:
                            nc.vector.tensor_add(
                                out=acc[:, span], in0=acc[:, span],
                                in1=pt[:, kb, span],
                            )
                    # single ones-matmul: partition-broadcast key sum
                    pd = bank(3, f"pd{h}_{rg}")
                    nc.tensor.matmul(pd, ones_sq, acc, start=True, stop=True)
                    # 1/denom as exp(-ln(denom)) on ACT: DVE reciprocal costs
                    # ~6.5ns/elem (3.3us/tile) and was stalling the PE
                    lnd = dinv_pool.tile([128, 512], F32, tag="lnd")
                    nc.scalar.activation(
                        out=lnd, in_=pd, func=mybir.ActivationFunctionType.Ln
                    )
                    dinv_b = dinv_pool.tile([128, 512], F32, tag="dinvb")
                    nc.scalar.activation(
                        out=dinv_b, in_=lnd,
                        func=mybir.ActivationFunctionType.Exp, scale=-1.0,
                    )
                    nc.vector.tensor_mul(
                        out=aT_sb[:, h, rows],
                        in0=po,
                        in1=dinv_b,
                    )

            # ---- per-head: attend head h, project q for head h+1 ----
            for h in range(HLOC):
                attention(h)
                if h + 1 < HLOC:
                    for tb in range(NTB):
                        proj_rope(h + 1, tb)

            # ---- output projection: out^T = wo_sh^T @ a^T ----
            # tb-outer so the tb=0 tiles (whose aT rows complete first) go
            # first (GPSIMD cannot read PSUM on this target, so DVE evacuates)
            # banks b5/b6/b7 are idle during the last head's rg=1 attention
            # (no next-head q projection), so the tb=0 tiles — which only
            # depend on every head's rg=0 rows — fill h7-rg1's exp stalls
            for tb in range(NTB):
                for et in range(KC):
                    pso = bank([5, 6, 7][et % 3], f"op{tb}_{et}")
                    for c in range(HLOC):
                        nc.tensor.matmul(
                            pso,
                            wo_sb[:, c, et * 128:(et + 1) * 128],
                            aT_sb[:, c, tb * 512:(tb + 1) * 512],
                            start=(c == 0),
                            stop=(c == HLOC - 1),
                        )
                    st = ostage.tile([128, 512], BF, tag="st")
                    nc.vector.tensor_copy(out=st, in_=pso)
                    nc.sync.dma_start(
                        out=outT_r[:, et, tb * 512:(tb + 1) * 512], in_=st
                    )

    return nc


LAST_RESULT = None
_TRACE = os.environ.get("BASS_ATTN_TRACE", "") == "1"

if _TRACE:
    # Register the NTFF profile hook that the agent image's antenv lacks
    # (test/profiling only; the graded path never enters this branch).
    try:
        import sys
        import types

        import antenv  # noqa: F401

        if "antenv.axon_hooks" not in sys.modules:
            _mod = types.ModuleType("antenv.axon_hooks")
            _hook_box = [None]
            _mod.set_axon_ntff_profile_hook = lambda h: _hook_box.__setitem__(0, h)
            _mod.get_axon_ntff_profile_hook = lambda: _hook_box[0]
            sys.modules["antenv.axon_hooks"] = _mod
            import antenv as _antenv

            _antenv.axon_hooks = _mod
            from trn_agent_boot.trn_boot import _ntff_profile_via_ctypes

            _mod.set_axon_ntff_profile_hook(
                _ntff_profile_via_ctypes("/opt/axon/libaxon_pjrt.so")
            )
    except Exception as e:  # pragma: no cover
        print(f"NTFF hook setup failed ({e}); tracing will be skipped")


def kernel(x, freqs_cis, wq, wk, wv, wo, seq_len=None, **_ignored):
    global _PROGRAM, LAST_RESULT
    x = np.ascontiguousarray(np.asarray(x, dtype=np.float32))
    fc = np.asarray(freqs_cis, dtype=np.float32)
    wq = np.asarray(wq, dtype=np.float32)
    wk = np.asarray(wk, dtype=np.float32)
    wv = np.asarray(wv, dtype=np.float32)
    wo = np.asarray(wo, dtype=np.float32)

    # host-side prep (sharding + transposed/bf16 views + rope/mask constants).
    # Head dims are permuted to halves layout (evens then odds) in wq/wk so
    # the on-device RoPE pair swap becomes a 64-partition block swap; scores
    # q'.k' are invariant to a shared permutation of the contracted dim.
    xT = np.ascontiguousarray(x.T).astype(BF16)                    # [2048, 4096]
    perm = np.concatenate([np.arange(0, D, 2), np.arange(1, D, 2)])  # [128]
    cs = fc[:S, :, 0].T.astype(np.float32)   # [64, S] cos per pair index
    sn = fc[:S, :, 1].T.astype(np.float32)   # [64, S]
    cos = np.ascontiguousarray(np.concatenate([cs, cs], axis=0)).astype(BF16)
    sin = np.ascontiguousarray(np.concatenate([-sn, sn], axis=0)).astype(BF16)
    k_idx = np.arange(128)[:, None]
    r_idx = np.arange(128)[None, :]
    tri = np.where(r_idx >= k_idx, 0.0, -1e9).astype(np.float32)

    def permheads(w):
        # apply the halves permutation within each 128-wide head block
        h = w.shape[1] // D
        return w.reshape(-1, h, D)[:, :, perm].reshape(w.shape)

    def pflat(w):
        # partition-major flat layout: out[p, kc*F+f] = w[kc*128+p, f]
        return np.ascontiguousarray(
            w.reshape(16, 128, -1).transpose(1, 0, 2).reshape(128, -1)
        )

    wq_p = permheads(wq)
    wk_p = permheads(wk)
    in_maps = []
    for c in range(NCORE):
        s, h2 = c // 2, c % 2
        wq_sh = wq_p[:, h2 * HLOC * D:(h2 + 1) * HLOC * D].astype(BF16)
        in_maps.append(
            {
                "xT": np.ascontiguousarray(xT[:, s * S:(s + 1) * S]),
                "wqh0": pflat(wq_sh[:, 0:D]),
                "wqr": np.ascontiguousarray(wq_sh[:, D:]),
                "wkf": pflat(wk_p[:, h2 * GLOC * D:(h2 + 1) * GLOC * D].astype(BF16)),
                "wvf": pflat(wv[:, h2 * GLOC * D:(h2 + 1) * GLOC * D].astype(BF16)),
                "wo": wo[h2 * HLOC * D:(h2 + 1) * HLOC * D, :].astype(BF16),
                "cosT": cos,
                "sinT": sin,
                "tri": tri,
            }
        )

    if _PROGRAM is None:
        _PROGRAM = _build_program()

    res = run_bass_kernel_spmd(
        _PROGRAM, in_maps, core_ids=list(range(NCORE)), trace=_TRACE
    )
    LAST_RESULT = res

    out = np.empty((B * S, DIM), np.float32)
    for s in range(B):
        outT = (
            res.results[2 * s]["outT"].astype(np.float32)
            + res.results[2 * s + 1]["outT"].astype(np.float32)
        )
        out[s * S:(s + 1) * S, :] = outT.T
    return out
